# revision 3
# baseline (speedup 1.0000x reference)
"""Trainium2 Bass kernel for per-sample conv self-attention.

Reference computation (per batch sample b, N = H*W = 4096, C = 64, C8 = 8):
    q = x @ wq + bq            [N, 8]
    k = x @ wk + bk            [N, 8]
    v = x @ wv + bv            [N, 64]
    attn = softmax(q @ k^T)    [N, N]   (softmax over keys, no scaling)
    out  = attn @ v * gamma + x

Sharding: data-parallel over batch — 8 samples onto 8 NeuronCores, one
sample per core.  Inside a core the attention matrix is processed
flash-style (never materialized in HBM):

  * x^T (with an appended ones-row for the biases) is built via PE
    transposes; q^T/k^T/v projections are small matmuls.
  * S^T blocks [m=128, n=512] are computed with K=8 matmuls packed 3-up
    into the 128x128 PE array via row groups (partition offsets 0/32/64),
    landing in PSUM banks.
  * exp() runs on ScalarE directly PSUM->SBUF (bf16 out) with a large
    free-dim (1536) to amortize per-instruction overhead.  No row-max
    subtraction is needed: |S| <= ~30 so exp stays well inside fp32/bf16
    range, matching the reference softmax up to rounding.
  * out^T [65, n] accumulates in PSUM as v'.T @ E^T where v' has a ones
    column appended - row 64 of out^T is then the softmax denominator.
  * Finale per n-block: cast to bf16, PE-transpose back to [n, 65],
    then VectorE computes y = out * (gamma/denom) + x and DMAs out.
"""

from contextlib import ExitStack

import numpy as np

import concourse.bass as bass
import concourse.mybir as mybir
import concourse.tile as tile
from concourse import bacc
from concourse.bass_utils import run_bass_kernel_spmd
from concourse.masks import make_identity

F32 = mybir.dt.float32
BF16 = mybir.dt.bfloat16
AF = mybir.ActivationFunctionType
ALU = mybir.AluOpType

B, H, W, C = 8, 64, 64, 64
N = H * W          # 4096 pixels (queries == keys)
C8 = C // 8        # 8  qk head dim
NB = 8             # n blocks
NBLK = N // NB     # 512 queries per block
MCH = N // 128     # 32 m-chunks of 128 keys
# m-chunks per group: 3 fit the 3 usable PE row-group replicas and give
# exp() free-dim 1536 (3 PSUM banks), double buffered: 6 banks + 1 out + 1 tp
GROUP_SIZES = [3] * 10 + [2]   # sums to 32


def _body(nc, tc, io):
    x_d, wq_d, bq_d, wk_d, bk_d, wv_d, bv_d, gamma_d, y_d = io


    # ---------------- persistent SBUF tensors ----------------
    ctx = ExitStack()
    singles = ctx.enter_context(tc.tile_pool(name="singles", bufs=1))
    ident = singles.tile([128, 128], BF16)       # identity for PE transposes
    x_sb = singles.tile([128, MCH * C], F32)     # resident x, chunk j at cols 64j
    xb = singles.tile([128, MCH * C], BF16)      # bf16 copy of x
    xT = singles.tile([C + 1, N], BF16)          # x^T with ones row 64
    qT_rep = singles.tile([128, N], BF16)        # q^T replicated at parts 0/32/64
    kT_rep = singles.tile([128, N], BF16)        # k^T replicated at parts 0/32/64
    v_all = singles.tile([128, MCH * (C + 1)], BF16)  # v'_j at cols 65j, ones col 64
    gamma_sb = singles.tile([128, 1], F32)
    wq_st = singles.tile([C + 1, C8], F32)
    wk_st = singles.tile([C + 1, C8], F32)
    wv_st = singles.tile([C + 1, C], F32)
    wqp = singles.tile([C + 1, 128], BF16)       # wq' replicated into cols 0/32/64
    wkp = singles.tile([C + 1, 128], BF16)
    wvp = singles.tile([C + 1, C], BF16)

    make_identity(nc, ident)

    # ---------------- input DMAs ----------------
    nc.sync.dma_start(
        out=x_sb[:].rearrange("p (c f) -> p c f", f=C),
        in_=x_d.rearrange("(c p) f -> p c f", p=128),
    )
    nc.sync.dma_start(out=wq_st[0:C, :], in_=wq_d)
    nc.sync.dma_start(out=wq_st[C : C + 1, :], in_=bq_d)
    nc.sync.dma_start(out=wk_st[0:C, :], in_=wk_d)
    nc.sync.dma_start(out=wk_st[C : C + 1, :], in_=bk_d)
    nc.sync.dma_start(out=wv_st[0:C, :], in_=wv_d)
    nc.sync.dma_start(out=wv_st[C : C + 1, :], in_=bv_d)
    nc.gpsimd.dma_start(out=gamma_sb[:], in_=gamma_d.to_broadcast((128, 1)))

    # weight staging -> bf16, replicated into PE column groups (zero padded)
    nc.gpsimd.memset(wqp[:], 0.0)
    nc.gpsimd.memset(wkp[:], 0.0)
    for i in range(3):
        nc.vector.tensor_copy(out=wqp[:, 32 * i : 32 * i + C8], in_=wq_st[:])
        nc.vector.tensor_copy(out=wkp[:, 32 * i : 32 * i + C8], in_=wk_st[:])
    nc.vector.tensor_copy(out=wvp[:], in_=wv_st[:])

    # ones row of x^T and ones columns of v'
    nc.gpsimd.memset(xT[C : C + 1, :], 1.0)
    nc.gpsimd.memset(
        v_all[:].rearrange("p (c f) -> p c f", f=C + 1)[:, :, C : C + 1], 1.0
    )

    # ---------------- preamble: transposes + projections ----------------
    with ExitStack() as pre:
        pre_tp = pre.enter_context(tc.tile_pool(name="pre_tp", bufs=2, space="PSUM"))
        pre_v = pre.enter_context(tc.tile_pool(name="pre_v", bufs=2, space="PSUM"))
        pre_pj = pre.enter_context(tc.tile_pool(name="pre_pj", bufs=1, space="PSUM"))

        # x -> bf16, then 32 PE transposes to build x^T
        nc.vector.tensor_copy(out=xb[:], in_=x_sb[:])
        for j in range(MCH):
            pt = pre_tp.tile([C, 128], BF16)
            nc.tensor.transpose(pt[:], xb[:, C * j : C * (j + 1)], ident[:])
            nc.vector.tensor_copy(out=xT[0:C, 128 * j : 128 * (j + 1)], in_=pt[:])

        # q^T / k^T projections: [65,128] weights x x'^T -> PSUM, cast+replicate
        for s in range(2):  # halves of n
            pq = pre_pj.tile([128, N // 2], F32)
            for b in range(4):
                nsl = slice(s * (N // 2) + b * NBLK, s * (N // 2) + (b + 1) * NBLK)
                nc.tensor.matmul(
                    pq[:, b * NBLK : (b + 1) * NBLK], wqp[:], xT[:, nsl],
                    start=True, stop=True,
                )
            nc.scalar.copy(
                out=qT_rep[:, s * (N // 2) : (s + 1) * (N // 2)], in_=pq[:]
            )
            pk = pre_pj.tile([128, N // 2], F32, tag="pq")
            for b in range(4):
                nsl = slice(s * (N // 2) + b * NBLK, s * (N // 2) + (b + 1) * NBLK)
                nc.tensor.matmul(
                    pk[:, b * NBLK : (b + 1) * NBLK], wkp[:], xT[:, nsl],
                    start=True, stop=True,
                )
            nc.vector.tensor_copy(
                out=kT_rep[:, s * (N // 2) : (s + 1) * (N // 2)], in_=pk[:]
            )

        # v projection per m-chunk: x_j @ wv' -> [128, 64], cast to bf16
        for j in range(MCH):
            pv = pre_v.tile([128, C], F32)
            nc.tensor.matmul(
                pv[:], xT[:, 128 * j : 128 * (j + 1)], wvp[:], start=True, stop=True
            )
            nc.vector.tensor_copy(
                out=v_all[:, (C + 1) * j : (C + 1) * j + C], in_=pv[:]
            )

    # ---------------- main loop ----------------
    st_pool = ctx.enter_context(tc.tile_pool(name="st", bufs=2, space="PSUM"))
    out_pool = ctx.enter_context(tc.tile_pool(name="oacc", bufs=1, space="PSUM"))
    ytp_pool = ctx.enter_context(tc.tile_pool(name="ytp", bufs=1, space="PSUM"))
    et_pool = ctx.enter_context(tc.tile_pool(name="et", bufs=3))
    ob_pool = ctx.enter_context(tc.tile_pool(name="ob", bufs=2))
    fin_pool = ctx.enter_context(tc.tile_pool(name="fin", bufs=2))

    iters = []
    for nb in range(NB):
        j0 = 0
        for gi, gs in enumerate(GROUP_SIZES):
            iters.append((nb, gi, list(range(j0, j0 + gs))))
            j0 += gs

    oaccs = {}
    pending_finales = []  # (due_idx, nb, ob_tile)

    def emit_s_exp(nb, chunks):
        gw = NBLK * len(chunks)
        st = st_pool.tile([128, gw], F32, tag="st")
        nsl = slice(nb * NBLK, (nb + 1) * NBLK)
        for i, j in enumerate(chunks):
            nc.tensor.matmul(
                st[:, i * NBLK : (i + 1) * NBLK],
                kT_rep[32 * i : 32 * i + C8, 128 * j : 128 * (j + 1)],
                qT_rep[32 * i : 32 * i + C8, nsl],
                start=True, stop=True,
            )
        et = et_pool.tile([128, gw], BF16, tag="et")
        nc.scalar.activation(out=et[:], in_=st[:], func=AF.Exp)
        return et

    def emit_pv(nb, gi, chunks, et):
        if gi == 0:
            oaccs[nb] = out_pool.tile([128, NBLK], F32, tag="oacc", name=f"oacc{nb}")
        oacc = oaccs[nb]
        for i, j in enumerate(chunks):
            nc.tensor.matmul(
                oacc[0 : C + 1, :],
                v_all[:, (C + 1) * j : (C + 1) * (j + 1)],
                et[:, i * NBLK : (i + 1) * NBLK],
                start=(j == 0), stop=(j == MCH - 1),
                skip_group_check=True,
            )

    def emit_finale(nb, ob):
        for k4 in range(4):
            yt = ytp_pool.tile([128, C + 1], BF16, tag="yt")
            nc.tensor.transpose(
                yt[:], ob[:, 128 * k4 : 128 * (k4 + 1)], ident[0 : C + 1, 0 : C + 1]
            )
            rc = fin_pool.tile([128, 1], F32, tag="rc")
            nc.vector.reciprocal(rc[:], yt[:, C : C + 1])
            yo = fin_pool.tile([128, C], F32, tag="yo")
            nc.vector.tensor_scalar(
                out=yo[:], in0=yt[:, 0:C], scalar1=rc[:], scalar2=gamma_sb[:],
                op0=ALU.mult, op1=ALU.mult,
            )
            ck = nb * 4 + k4
            nc.vector.tensor_add(yo[:], yo[:], x_sb[:, C * ck : C * (ck + 1)])
            nc.sync.dma_start(out=y_d[128 * ck : 128 * (ck + 1), :], in_=yo[:])

    prev = None
    for idx, (nb, gi, chunks) in enumerate(iters):
        while pending_finales and pending_finales[0][0] <= idx:
            _, fnb, fob = pending_finales.pop(0)
            emit_finale(fnb, fob)
        et = emit_s_exp(nb, chunks)
        if prev is not None:
            pnb, pgi, pchunks, pet = prev
            emit_pv(pnb, pgi, pchunks, pet)
            if pgi == len(GROUP_SIZES) - 1:
                ob = ob_pool.tile([C + 1, NBLK], BF16, tag="ob")
                nc.vector.tensor_copy(out=ob[:], in_=oaccs[pnb][0 : C + 1, :])
                pending_finales.append((idx + 2, pnb, ob))
        prev = (nb, gi, chunks, et)

    pnb, pgi, pchunks, pet = prev
    emit_pv(pnb, pgi, pchunks, pet)
    ob = ob_pool.tile([C + 1, NBLK], BF16, tag="ob")
    nc.vector.tensor_copy(out=ob[:], in_=oaccs[pnb][0 : C + 1, :])
    pending_finales.append((0, pnb, ob))
    for _, fnb, fob in pending_finales:
        emit_finale(fnb, fob)

    ctx.close()


def build_program():
    nc = bacc.Bacc("TRN2", target_bir_lowering=False, debug=False, num_devices=8)
    x_d = nc.dram_tensor("x", [N, C], F32, kind="ExternalInput").ap()
    wq_d = nc.dram_tensor("wq", [C, C8], F32, kind="ExternalInput").ap()
    bq_d = nc.dram_tensor("bq", [1, C8], F32, kind="ExternalInput").ap()
    wk_d = nc.dram_tensor("wk", [C, C8], F32, kind="ExternalInput").ap()
    bk_d = nc.dram_tensor("bk", [1, C8], F32, kind="ExternalInput").ap()
    wv_d = nc.dram_tensor("wv", [C, C], F32, kind="ExternalInput").ap()
    bv_d = nc.dram_tensor("bv", [1, C], F32, kind="ExternalInput").ap()
    gamma_d = nc.dram_tensor("gamma", [1, 1], F32, kind="ExternalInput").ap()
    y_d = nc.dram_tensor("y", [N, C], F32, kind="ExternalOutput").ap()

    io = (x_d, wq_d, bq_d, wk_d, bk_d, wv_d, bv_d, gamma_d, y_d)
    with tile.TileContext(nc) as tc:
        _body(nc, tc, io)
    nc.compile()
    return nc


_CACHE = {}


def _get_program():
    if "nc" not in _CACHE:
        _CACHE["nc"] = build_program()
    return _CACHE["nc"]


def make_in_maps(inputs):
    x = np.ascontiguousarray(np.asarray(inputs["x"], dtype=np.float32))
    wq = np.ascontiguousarray(np.asarray(inputs["wq"], dtype=np.float32))
    bq = np.asarray(inputs["bq"], dtype=np.float32).reshape(1, C8)
    wk = np.ascontiguousarray(np.asarray(inputs["wk"], dtype=np.float32))
    bk = np.asarray(inputs["bk"], dtype=np.float32).reshape(1, C8)
    wv = np.ascontiguousarray(np.asarray(inputs["wv"], dtype=np.float32))
    bv = np.asarray(inputs["bv"], dtype=np.float32).reshape(1, C)
    gamma = np.asarray(inputs["gamma"], dtype=np.float32).reshape(1, 1)
    in_maps = []
    for b in range(B):
        in_maps.append(
            {
                "x": np.ascontiguousarray(x[b].reshape(N, C)),
                "wq": wq, "bq": bq, "wk": wk, "bk": bk,
                "wv": wv, "bv": bv, "gamma": gamma,
            }
        )
    return in_maps


def run(inputs, **kwargs):
    nc = _get_program()
    res = run_bass_kernel_spmd(
        nc, make_in_maps(inputs), core_ids=list(range(B)), **kwargs
    )
    y = np.stack([res.results[b]["y"] for b in range(B)], axis=0)
    return y.reshape(B, H, W, C).astype(np.float32), res


def kernel(**inputs) -> np.ndarray:
    y, _ = run(inputs)
    return y


# revision 5
# speedup vs baseline: 1.0088x; 1.0088x over previous
"""Trainium2 Bass kernel for per-sample conv self-attention.

Reference computation (per batch sample b, N = H*W = 4096, C = 64, C8 = 8):
    q = x @ wq + bq            [N, 8]
    k = x @ wk + bk            [N, 8]
    v = x @ wv + bv            [N, 64]
    attn = softmax(q @ k^T)    [N, N]   (softmax over keys, no scaling)
    out  = attn @ v * gamma + x

Sharding: data-parallel over batch — 8 samples onto 8 NeuronCores, one
sample per core.  Inside a core the attention matrix is processed
flash-style (never materialized in HBM):

  * x^T (with an appended ones-row for the biases) is built via PE
    transposes; q^T/k^T/v projections are small matmuls.
  * S^T blocks [m=128, n=512] are computed with K=8 matmuls packed 3-up
    into the 128x128 PE array via row groups (partition offsets 0/32/64),
    landing in PSUM banks.
  * exp() runs on ScalarE directly PSUM->SBUF (bf16 out) with a large
    free-dim (1536) to amortize per-instruction overhead.  No row-max
    subtraction is needed: |S| <= ~30 so exp stays well inside fp32/bf16
    range, matching the reference softmax up to rounding.
  * out^T [65, n] accumulates in PSUM as v'.T @ E^T where v' has a ones
    column appended - row 64 of out^T is then the softmax denominator.
  * Finale per n-block: cast to bf16, PE-transpose back to [n, 65],
    then VectorE computes y = out * (gamma/denom) + x and DMAs out.
"""

from contextlib import ExitStack

import numpy as np

import concourse.bass as bass
import concourse.mybir as mybir
import concourse.tile as tile
from concourse import bacc
from concourse.bass_utils import run_bass_kernel_spmd
from concourse.masks import make_identity

F32 = mybir.dt.float32
BF16 = mybir.dt.bfloat16
AF = mybir.ActivationFunctionType
ALU = mybir.AluOpType

B, H, W, C = 8, 64, 64, 64
N = H * W          # 4096 pixels (queries == keys)
C8 = C // 8        # 8  qk head dim
NB = 8             # n blocks
NBLK = N // NB     # 512 queries per block
MCH = N // 128     # 32 m-chunks of 128 keys
# m-chunks per group: 3 fit the 3 usable PE row-group replicas and give
# exp() free-dim 1536 (3 PSUM banks), double buffered: 6 banks + 1 out + 1 tp
GROUP_SIZES = [3] * 10 + [2]   # sums to 32


def _body(nc, tc, io):
    x_d, wq_d, bq_d, wk_d, bk_d, wv_d, bv_d, gamma_d, y_d = io


    # ---------------- persistent SBUF tensors ----------------
    ctx = ExitStack()
    singles = ctx.enter_context(tc.tile_pool(name="singles", bufs=1))
    ident = singles.tile([128, 128], BF16)       # identity for PE transposes
    x_sb = singles.tile([128, MCH * C], F32)     # resident x, chunk j at cols 64j
    xb = singles.tile([128, MCH * C], BF16)      # bf16 copy of x
    xT = singles.tile([C + 1, N], BF16)          # x^T with ones row 64
    qT_rep = singles.tile([128, N], BF16)        # q^T replicated at parts 0/32/64
    kT_rep = singles.tile([128, N], BF16)        # k^T replicated at parts 0/32/64
    v_all = singles.tile([128, MCH * (C + 1)], BF16)  # v'_j at cols 65j, ones col 64
    gamma_sb = singles.tile([128, 1], F32)
    wq_st = singles.tile([C + 1, C8], F32)
    wk_st = singles.tile([C + 1, C8], F32)
    wv_st = singles.tile([C + 1, C], F32)
    wqp = singles.tile([C + 1, 128], BF16)       # wq' replicated into cols 0/32/64
    wkp = singles.tile([C + 1, 128], BF16)
    wvp = singles.tile([C + 1, C], BF16)

    make_identity(nc, ident)

    # ---------------- input DMAs ----------------
    nc.sync.dma_start(out=wq_st[0:C, :], in_=wq_d)
    nc.sync.dma_start(out=wq_st[C : C + 1, :], in_=bq_d)
    nc.sync.dma_start(out=wk_st[0:C, :], in_=wk_d)
    nc.sync.dma_start(out=wk_st[C : C + 1, :], in_=bk_d)
    nc.sync.dma_start(out=wv_st[0:C, :], in_=wv_d)
    nc.sync.dma_start(out=wv_st[C : C + 1, :], in_=bv_d)
    nc.gpsimd.dma_start(out=gamma_sb[:], in_=gamma_d.to_broadcast((128, 1)))

    # weight staging -> bf16, replicated into PE column groups (zero padded)
    nc.gpsimd.memset(wqp[:], 0.0)
    nc.gpsimd.memset(wkp[:], 0.0)
    for i in range(3):
        nc.vector.tensor_copy(out=wqp[:, 32 * i : 32 * i + C8], in_=wq_st[:])
        nc.vector.tensor_copy(out=wkp[:, 32 * i : 32 * i + C8], in_=wk_st[:])
    nc.vector.tensor_copy(out=wvp[:], in_=wv_st[:])

    # ones row of x^T and ones columns of v'
    nc.gpsimd.memset(xT[C : C + 1, :], 1.0)
    nc.gpsimd.memset(
        v_all[:].rearrange("p (c f) -> p c f", f=C + 1)[:, :, C : C + 1], 1.0
    )

    # ---------------- preamble: transposes + projections ----------------
    # Pipelined in two n-halves so the main loop can start after half 0:
    # x DMA half -> bf16 cast -> PE transposes -> q/k projections (1024-wide
    # rounds, double buffered) -> v projections.  q casts go on ScalarE while
    # k casts go on VectorE so the two chains run concurrently.
    with ExitStack() as pre:
        pre_tp = pre.enter_context(tc.tile_pool(name="pre_tp", bufs=2, space="PSUM"))
        pre_v = pre.enter_context(tc.tile_pool(name="pre_v", bufs=2, space="PSUM"))
        pre_pj = pre.enter_context(tc.tile_pool(name="pre_pj", bufs=2, space="PSUM"))

        HN = N // 2
        for h in range(2):
            # x rows for m-chunks 16h .. 16h+15
            nc.sync.dma_start(
                out=x_sb[:, 1024 * h : 1024 * (h + 1)].rearrange(
                    "p (c f) -> p c f", f=C
                ),
                in_=x_d[2048 * h : 2048 * (h + 1), :].rearrange(
                    "(c p) f -> p c f", p=128
                ),
            )
            nc.vector.tensor_copy(
                out=xb[:, 1024 * h : 1024 * (h + 1)],
                in_=x_sb[:, 1024 * h : 1024 * (h + 1)],
            )
            for j in range(16 * h, 16 * h + 16):
                pt = pre_tp.tile([C, 128], BF16, tag="pt")
                nc.tensor.transpose(pt[:], xb[:, C * j : C * (j + 1)], ident[:])
                nc.vector.tensor_copy(
                    out=xT[0:C, 128 * j : 128 * (j + 1)], in_=pt[:]
                )
            # q/k projections over this half, two 1024-wide rounds each
            for r in range(2):
                csl = slice(HN * h + 1024 * r, HN * h + 1024 * (r + 1))
                pq = pre_pj.tile([128, 1024], F32, tag="pj")
                for b2 in range(2):
                    bsl = slice(
                        HN * h + 1024 * r + 512 * b2,
                        HN * h + 1024 * r + 512 * (b2 + 1),
                    )
                    nc.tensor.matmul(
                        pq[:, 512 * b2 : 512 * (b2 + 1)], wqp[:], xT[:, bsl],
                        start=True, stop=True,
                    )
                nc.scalar.copy(out=qT_rep[:, csl], in_=pq[:])
                pk = pre_pj.tile([128, 1024], F32, tag="pj")
                for b2 in range(2):
                    bsl = slice(
                        HN * h + 1024 * r + 512 * b2,
                        HN * h + 1024 * r + 512 * (b2 + 1),
                    )
                    nc.tensor.matmul(
                        pk[:, 512 * b2 : 512 * (b2 + 1)], wkp[:], xT[:, bsl],
                        start=True, stop=True,
                    )
                nc.vector.tensor_copy(out=kT_rep[:, csl], in_=pk[:])
            # v projections: first 6 chunks right after half 0 (needed by the
            # first two main-loop groups), the rest after half 1
            vr = range(0, 6) if h == 0 else range(6, MCH)
            for j in vr:
                pv = pre_v.tile([128, C], F32, tag="pv")
                nc.tensor.matmul(
                    pv[:], xT[:, 128 * j : 128 * (j + 1)], wvp[:],
                    start=True, stop=True,
                )
                nc.vector.tensor_copy(
                    out=v_all[:, (C + 1) * j : (C + 1) * j + C], in_=pv[:]
                )

    # ---------------- main loop ----------------
    st_pool = ctx.enter_context(tc.tile_pool(name="st", bufs=2, space="PSUM"))
    out_pool = ctx.enter_context(tc.tile_pool(name="oacc", bufs=1, space="PSUM"))
    ytp_pool = ctx.enter_context(tc.tile_pool(name="ytp", bufs=1, space="PSUM"))
    et_pool = ctx.enter_context(tc.tile_pool(name="et", bufs=3))
    ob_pool = ctx.enter_context(tc.tile_pool(name="ob", bufs=2))
    fin_pool = ctx.enter_context(tc.tile_pool(name="fin", bufs=2))

    iters = []
    for nb in range(NB):
        j0 = 0
        for gi, gs in enumerate(GROUP_SIZES):
            iters.append((nb, gi, list(range(j0, j0 + gs))))
            j0 += gs

    oaccs = {}
    pending_finales = []  # (due_idx, nb, ob_tile)

    def emit_s_exp(nb, chunks):
        gw = NBLK * len(chunks)
        st = st_pool.tile([128, gw], F32, tag="st")
        nsl = slice(nb * NBLK, (nb + 1) * NBLK)
        for i, j in enumerate(chunks):
            nc.tensor.matmul(
                st[:, i * NBLK : (i + 1) * NBLK],
                kT_rep[32 * i : 32 * i + C8, 128 * j : 128 * (j + 1)],
                qT_rep[32 * i : 32 * i + C8, nsl],
                start=True, stop=True,
            )
        et = et_pool.tile([128, gw], BF16, tag="et")
        nc.scalar.activation(out=et[:], in_=st[:], func=AF.Exp)
        return et

    def emit_pv(nb, gi, chunks, et):
        if gi == 0:
            oaccs[nb] = out_pool.tile([128, NBLK], F32, tag="oacc", name=f"oacc{nb}")
        oacc = oaccs[nb]
        for i, j in enumerate(chunks):
            nc.tensor.matmul(
                oacc[0 : C + 1, :],
                v_all[:, (C + 1) * j : (C + 1) * (j + 1)],
                et[:, i * NBLK : (i + 1) * NBLK],
                start=(j == 0), stop=(j == MCH - 1),
                skip_group_check=True,
            )

    def emit_finale(nb, ob, k4):
        if True:
            yt = ytp_pool.tile([128, C + 1], BF16, tag="yt")
            nc.tensor.transpose(
                yt[:], ob[:, 128 * k4 : 128 * (k4 + 1)], ident[0 : C + 1, 0 : C + 1]
            )
            rc = fin_pool.tile([128, 1], F32, tag="rc")
            nc.vector.reciprocal(rc[:], yt[:, C : C + 1])
            yo = fin_pool.tile([128, C], F32, tag="yo")
            nc.vector.tensor_scalar(
                out=yo[:], in0=yt[:, 0:C], scalar1=rc[:], scalar2=gamma_sb[:],
                op0=ALU.mult, op1=ALU.mult,
            )
            ck = nb * 4 + k4
            nc.vector.tensor_add(yo[:], yo[:], x_sb[:, C * ck : C * (ck + 1)])
            nc.sync.dma_start(out=y_d[128 * ck : 128 * (ck + 1), :], in_=yo[:])

    prev = None
    for idx, (nb, gi, chunks) in enumerate(iters):
        while pending_finales and pending_finales[0][0] <= idx:
            _, fnb, fob, fk4 = pending_finales.pop(0)
            emit_finale(fnb, fob, fk4)
        et = emit_s_exp(nb, chunks)
        if prev is not None:
            pnb, pgi, pchunks, pet = prev
            emit_pv(pnb, pgi, pchunks, pet)
            if pgi == len(GROUP_SIZES) - 1:
                ob = ob_pool.tile([C + 1, NBLK], BF16, tag="ob")
                nc.vector.tensor_copy(out=ob[:], in_=oaccs[pnb][0 : C + 1, :])
                for t in range(4):
                    pending_finales.append((idx + 2 + t, pnb, ob, t))
        prev = (nb, gi, chunks, et)

    pnb, pgi, pchunks, pet = prev
    emit_pv(pnb, pgi, pchunks, pet)
    ob = ob_pool.tile([C + 1, NBLK], BF16, tag="ob")
    nc.vector.tensor_copy(out=ob[:], in_=oaccs[pnb][0 : C + 1, :])
    for t in range(4):
        pending_finales.append((0, pnb, ob, t))
    for _, fnb, fob, fk4 in pending_finales:
        emit_finale(fnb, fob, fk4)

    ctx.close()


def build_program():
    nc = bacc.Bacc("TRN2", target_bir_lowering=False, debug=False, num_devices=8)
    x_d = nc.dram_tensor("x", [N, C], F32, kind="ExternalInput").ap()
    wq_d = nc.dram_tensor("wq", [C, C8], F32, kind="ExternalInput").ap()
    bq_d = nc.dram_tensor("bq", [1, C8], F32, kind="ExternalInput").ap()
    wk_d = nc.dram_tensor("wk", [C, C8], F32, kind="ExternalInput").ap()
    bk_d = nc.dram_tensor("bk", [1, C8], F32, kind="ExternalInput").ap()
    wv_d = nc.dram_tensor("wv", [C, C], F32, kind="ExternalInput").ap()
    bv_d = nc.dram_tensor("bv", [1, C], F32, kind="ExternalInput").ap()
    gamma_d = nc.dram_tensor("gamma", [1, 1], F32, kind="ExternalInput").ap()
    y_d = nc.dram_tensor("y", [N, C], F32, kind="ExternalOutput").ap()

    io = (x_d, wq_d, bq_d, wk_d, bk_d, wv_d, bv_d, gamma_d, y_d)
    with tile.TileContext(nc) as tc:
        _body(nc, tc, io)
    nc.compile()
    return nc


_CACHE = {}


def _get_program():
    if "nc" not in _CACHE:
        _CACHE["nc"] = build_program()
    return _CACHE["nc"]


def make_in_maps(inputs):
    x = np.ascontiguousarray(np.asarray(inputs["x"], dtype=np.float32))
    wq = np.ascontiguousarray(np.asarray(inputs["wq"], dtype=np.float32))
    bq = np.asarray(inputs["bq"], dtype=np.float32).reshape(1, C8)
    wk = np.ascontiguousarray(np.asarray(inputs["wk"], dtype=np.float32))
    bk = np.asarray(inputs["bk"], dtype=np.float32).reshape(1, C8)
    wv = np.ascontiguousarray(np.asarray(inputs["wv"], dtype=np.float32))
    bv = np.asarray(inputs["bv"], dtype=np.float32).reshape(1, C)
    gamma = np.asarray(inputs["gamma"], dtype=np.float32).reshape(1, 1)
    in_maps = []
    for b in range(B):
        in_maps.append(
            {
                "x": np.ascontiguousarray(x[b].reshape(N, C)),
                "wq": wq, "bq": bq, "wk": wk, "bk": bk,
                "wv": wv, "bv": bv, "gamma": gamma,
            }
        )
    return in_maps


def run(inputs, **kwargs):
    nc = _get_program()
    res = run_bass_kernel_spmd(
        nc, make_in_maps(inputs), core_ids=list(range(B)), **kwargs
    )
    y = np.stack([res.results[b]["y"] for b in range(B)], axis=0)
    return y.reshape(B, H, W, C).astype(np.float32), res


def kernel(**inputs) -> np.ndarray:
    y, _ = run(inputs)
    return y


# revision 6
# speedup vs baseline: 1.0535x; 1.0443x over previous
"""Trainium2 Bass kernel for per-sample conv self-attention.

Reference computation (per batch sample b, N = H*W = 4096, C = 64, C8 = 8):
    q = x @ wq + bq            [N, 8]
    k = x @ wk + bk            [N, 8]
    v = x @ wv + bv            [N, 64]
    attn = softmax(q @ k^T)    [N, N]   (softmax over keys, no scaling)
    out  = attn @ v * gamma + x

Sharding: data-parallel over batch — 8 samples onto 8 NeuronCores, one
sample per core.  Inside a core the attention matrix is processed
flash-style (never materialized in HBM):

  * x^T (with an appended ones-row for the biases) is built via PE
    transposes; q^T/k^T/v projections are small matmuls.
  * S^T blocks [m=128, n=512] are computed with K=8 matmuls packed 3-up
    into the 128x128 PE array via row groups (partition offsets 0/32/64),
    landing in PSUM banks.
  * exp() runs on ScalarE directly PSUM->SBUF (bf16 out) with a large
    free-dim (1536) to amortize per-instruction overhead.  No row-max
    subtraction is needed: |S| <= ~30 so exp stays well inside fp32/bf16
    range, matching the reference softmax up to rounding.
  * out^T [65, n] accumulates in PSUM as v'.T @ E^T where v' has a ones
    column appended - row 64 of out^T is then the softmax denominator.
  * Finale per n-block: cast to bf16, PE-transpose back to [n, 65],
    then VectorE computes y = out * (gamma/denom) + x and DMAs out.
"""

from contextlib import ExitStack

import numpy as np

import concourse.bass as bass
import concourse.mybir as mybir
import concourse.tile as tile
from concourse import bacc
from concourse.bass_utils import run_bass_kernel_spmd
from concourse.masks import make_identity

F32 = mybir.dt.float32
BF16 = mybir.dt.bfloat16
AF = mybir.ActivationFunctionType
ALU = mybir.AluOpType

B, H, W, C = 8, 64, 64, 64
N = H * W          # 4096 pixels (queries == keys)
C8 = C // 8        # 8  qk head dim
NB = 8             # n blocks
NBLK = N // NB     # 512 queries per block
MCH = N // 128     # 32 m-chunks of 128 keys
# m-chunks per group: 3 fit the 3 usable PE row-group replicas and give
# exp() free-dim 1536 (3 PSUM banks), double buffered: 6 banks + 1 out + 1 tp
GROUP_SIZES = [3] * 10 + [2]   # sums to 32


def _body(nc, tc, io):
    x_d, wq_d, bq_d, wk_d, bk_d, wv_d, bv_d, gamma_d, y_d = io


    # ---------------- persistent SBUF tensors ----------------
    ctx = ExitStack()
    singles = ctx.enter_context(tc.tile_pool(name="singles", bufs=1))
    ident = singles.tile([128, 128], BF16)       # identity for PE transposes
    x_sb = singles.tile([128, MCH * C], F32)     # resident x, chunk j at cols 64j
    xb = singles.tile([128, MCH * C], BF16)      # bf16 copy of x
    xT = singles.tile([C + 1, N], BF16)          # x^T with ones row 64
    qT_rep = singles.tile([128, N], BF16)        # q^T replicated at parts 0/32/64
    kT_rep = singles.tile([128, N], BF16)        # k^T replicated at parts 0/32/64
    v_all = singles.tile([128, MCH * (C + 1)], BF16)  # v'_j at cols 65j, ones col 64
    gamma_sb = singles.tile([128, 1], F32)
    wq_st = singles.tile([C + 1, C8], F32)
    wk_st = singles.tile([C + 1, C8], F32)
    wv_st = singles.tile([C + 1, C], F32)
    wqp = singles.tile([C + 1, 128], BF16)       # wq' replicated into cols 0/32/64
    wkp = singles.tile([C + 1, 128], BF16)
    wvp = singles.tile([C + 1, C], BF16)

    make_identity(nc, ident)

    # ---------------- input DMAs ----------------
    nc.sync.dma_start(out=wq_st[0:C, :], in_=wq_d)
    nc.sync.dma_start(out=wq_st[C : C + 1, :], in_=bq_d)
    nc.sync.dma_start(out=wk_st[0:C, :], in_=wk_d)
    nc.sync.dma_start(out=wk_st[C : C + 1, :], in_=bk_d)
    nc.sync.dma_start(out=wv_st[0:C, :], in_=wv_d)
    nc.sync.dma_start(out=wv_st[C : C + 1, :], in_=bv_d)
    nc.gpsimd.dma_start(out=gamma_sb[:], in_=gamma_d.to_broadcast((128, 1)))

    # weight staging -> bf16, replicated into PE column groups (zero padded)
    nc.gpsimd.memset(wqp[:], 0.0)
    nc.gpsimd.memset(wkp[:], 0.0)
    for i in range(3):
        nc.vector.tensor_copy(out=wqp[:, 32 * i : 32 * i + C8], in_=wq_st[:])
        nc.vector.tensor_copy(out=wkp[:, 32 * i : 32 * i + C8], in_=wk_st[:])
    nc.vector.tensor_copy(out=wvp[:], in_=wv_st[:])

    # ones row of x^T and ones columns of v'
    nc.gpsimd.memset(xT[C : C + 1, :], 1.0)
    nc.gpsimd.memset(
        v_all[:].rearrange("p (c f) -> p c f", f=C + 1)[:, :, C : C + 1], 1.0
    )

    # ---------------- preamble: transposes + projections ----------------
    # Pipelined in two n-halves so the main loop can start after half 0:
    # x DMA half -> bf16 cast -> PE transposes -> q/k projections (1024-wide
    # rounds, double buffered) -> v projections.  q casts go on ScalarE while
    # k casts go on VectorE so the two chains run concurrently.
    with ExitStack() as pre:
        pre_tp = pre.enter_context(tc.tile_pool(name="pre_tp", bufs=2, space="PSUM"))
        pre_v = pre.enter_context(tc.tile_pool(name="pre_v", bufs=2, space="PSUM"))
        pre_pj = pre.enter_context(tc.tile_pool(name="pre_pj", bufs=2, space="PSUM"))

        HN = N // 2
        # both halves: DMA -> cast -> transposes (dense PE stream, no ladder)
        for h in range(2):
            nc.sync.dma_start(
                out=x_sb[:, 1024 * h : 1024 * (h + 1)].rearrange(
                    "p (c f) -> p c f", f=C
                ),
                in_=x_d[2048 * h : 2048 * (h + 1), :].rearrange(
                    "(c p) f -> p c f", p=128
                ),
            )
            nc.vector.tensor_copy(
                out=xb[:, 1024 * h : 1024 * (h + 1)],
                in_=x_sb[:, 1024 * h : 1024 * (h + 1)],
            )
            for j in range(16 * h, 16 * h + 16):
                pt = pre_tp.tile([C, 128], BF16, tag="pt")
                nc.tensor.transpose(pt[:], xb[:, C * j : C * (j + 1)], ident[:])
                nc.vector.tensor_copy(
                    out=xT[0:C, 128 * j : 128 * (j + 1)], in_=pt[:]
                )
        # q/k projections, 1024-wide double-buffered rounds; q casts on
        # ScalarE, k casts on VectorE so the chains run concurrently
        for r in range(4):
            csl = slice(1024 * r, 1024 * (r + 1))
            pq = pre_pj.tile([128, 1024], F32, tag="pj")
            for b2 in range(2):
                bsl = slice(1024 * r + 512 * b2, 1024 * r + 512 * (b2 + 1))
                nc.tensor.matmul(
                    pq[:, 512 * b2 : 512 * (b2 + 1)], wqp[:], xT[:, bsl],
                    start=True, stop=True,
                )
            nc.scalar.copy(out=qT_rep[:, csl], in_=pq[:])
            pk = pre_pj.tile([128, 1024], F32, tag="pj")
            for b2 in range(2):
                bsl = slice(1024 * r + 512 * b2, 1024 * r + 512 * (b2 + 1))
                nc.tensor.matmul(
                    pk[:, 512 * b2 : 512 * (b2 + 1)], wkp[:], xT[:, bsl],
                    start=True, stop=True,
                )
            nc.vector.tensor_copy(out=kT_rep[:, csl], in_=pk[:])
        # v projections
        for j in range(MCH):
            pv = pre_v.tile([128, C], F32, tag="pv")
            nc.tensor.matmul(
                pv[:], xT[:, 128 * j : 128 * (j + 1)], wvp[:],
                start=True, stop=True,
            )
            nc.vector.tensor_copy(
                out=v_all[:, (C + 1) * j : (C + 1) * j + C], in_=pv[:]
            )

    # ---------------- main loop ----------------
    st_pool = ctx.enter_context(tc.tile_pool(name="st", bufs=2, space="PSUM"))
    out_pool = ctx.enter_context(tc.tile_pool(name="oacc", bufs=1, space="PSUM"))
    ytp_pool = ctx.enter_context(tc.tile_pool(name="ytp", bufs=1, space="PSUM"))
    et_pool = ctx.enter_context(tc.tile_pool(name="et", bufs=3))
    ob_pool = ctx.enter_context(tc.tile_pool(name="ob", bufs=2))
    fin_pool = ctx.enter_context(tc.tile_pool(name="fin", bufs=2))

    iters = []
    for nb in range(NB):
        j0 = 0
        for gi, gs in enumerate(GROUP_SIZES):
            iters.append((nb, gi, list(range(j0, j0 + gs))))
            j0 += gs

    oaccs = {}
    pending_finales = []  # (due_idx, nb, ob_tile)

    def emit_s_exp(nb, chunks):
        gw = NBLK * len(chunks)
        st = st_pool.tile([128, gw], F32, tag="st")
        nsl = slice(nb * NBLK, (nb + 1) * NBLK)
        for i, j in enumerate(chunks):
            nc.tensor.matmul(
                st[:, i * NBLK : (i + 1) * NBLK],
                kT_rep[32 * i : 32 * i + C8, 128 * j : 128 * (j + 1)],
                qT_rep[32 * i : 32 * i + C8, nsl],
                start=True, stop=True,
            )
        et = et_pool.tile([128, gw], BF16, tag="et")
        nc.scalar.activation(out=et[:], in_=st[:], func=AF.Exp)
        return et

    def emit_pv(nb, gi, chunks, et):
        if gi == 0:
            oaccs[nb] = out_pool.tile([128, NBLK], F32, tag="oacc", name=f"oacc{nb}")
        oacc = oaccs[nb]
        for i, j in enumerate(chunks):
            nc.tensor.matmul(
                oacc[0 : C + 1, :],
                v_all[:, (C + 1) * j : (C + 1) * (j + 1)],
                et[:, i * NBLK : (i + 1) * NBLK],
                start=(j == 0), stop=(j == MCH - 1),
                skip_group_check=True,
            )

    def emit_finale(nb, ob, k4):
        if True:
            yt = ytp_pool.tile([128, C + 1], BF16, tag="yt")
            nc.tensor.transpose(
                yt[:], ob[:, 128 * k4 : 128 * (k4 + 1)], ident[0 : C + 1, 0 : C + 1]
            )
            rc = fin_pool.tile([128, 1], F32, tag="rc")
            nc.vector.reciprocal(rc[:], yt[:, C : C + 1])
            yo = fin_pool.tile([128, C], F32, tag="yo")
            nc.vector.tensor_scalar(
                out=yo[:], in0=yt[:, 0:C], scalar1=rc[:], scalar2=gamma_sb[:],
                op0=ALU.mult, op1=ALU.mult,
            )
            ck = nb * 4 + k4
            nc.vector.tensor_add(yo[:], yo[:], x_sb[:, C * ck : C * (ck + 1)])
            nc.sync.dma_start(out=y_d[128 * ck : 128 * (ck + 1), :], in_=yo[:])

    prevs = []
    for idx, (nb, gi, chunks) in enumerate(iters):
        while pending_finales and pending_finales[0][0] <= idx:
            _, fnb, fob, fk4 = pending_finales.pop(0)
            emit_finale(fnb, fob, fk4)
        et = emit_s_exp(nb, chunks)
        prevs.append((nb, gi, chunks, et))
        if len(prevs) > 2:
            pnb, pgi, pchunks, pet = prevs.pop(0)
            emit_pv(pnb, pgi, pchunks, pet)
            if pgi == len(GROUP_SIZES) - 1:
                ob = ob_pool.tile([C + 1, NBLK], BF16, tag="ob")
                nc.vector.tensor_copy(out=ob[:], in_=oaccs[pnb][0 : C + 1, :])
                for t in range(4):
                    pending_finales.append((idx + 1 + t, pnb, ob, t))

    for pnb, pgi, pchunks, pet in prevs:
        emit_pv(pnb, pgi, pchunks, pet)
        if pgi == len(GROUP_SIZES) - 1:
            ob = ob_pool.tile([C + 1, NBLK], BF16, tag="ob")
            nc.vector.tensor_copy(out=ob[:], in_=oaccs[pnb][0 : C + 1, :])
            for t in range(4):
                pending_finales.append((0, pnb, ob, t))
    for _, fnb, fob, fk4 in pending_finales:
        emit_finale(fnb, fob, fk4)

    ctx.close()


def build_program():
    nc = bacc.Bacc("TRN2", target_bir_lowering=False, debug=False, num_devices=8)
    x_d = nc.dram_tensor("x", [N, C], F32, kind="ExternalInput").ap()
    wq_d = nc.dram_tensor("wq", [C, C8], F32, kind="ExternalInput").ap()
    bq_d = nc.dram_tensor("bq", [1, C8], F32, kind="ExternalInput").ap()
    wk_d = nc.dram_tensor("wk", [C, C8], F32, kind="ExternalInput").ap()
    bk_d = nc.dram_tensor("bk", [1, C8], F32, kind="ExternalInput").ap()
    wv_d = nc.dram_tensor("wv", [C, C], F32, kind="ExternalInput").ap()
    bv_d = nc.dram_tensor("bv", [1, C], F32, kind="ExternalInput").ap()
    gamma_d = nc.dram_tensor("gamma", [1, 1], F32, kind="ExternalInput").ap()
    y_d = nc.dram_tensor("y", [N, C], F32, kind="ExternalOutput").ap()

    io = (x_d, wq_d, bq_d, wk_d, bk_d, wv_d, bv_d, gamma_d, y_d)
    with tile.TileContext(nc) as tc:
        _body(nc, tc, io)
    nc.compile()
    return nc


_CACHE = {}


def _get_program():
    if "nc" not in _CACHE:
        _CACHE["nc"] = build_program()
    return _CACHE["nc"]


def make_in_maps(inputs):
    x = np.ascontiguousarray(np.asarray(inputs["x"], dtype=np.float32))
    wq = np.ascontiguousarray(np.asarray(inputs["wq"], dtype=np.float32))
    bq = np.asarray(inputs["bq"], dtype=np.float32).reshape(1, C8)
    wk = np.ascontiguousarray(np.asarray(inputs["wk"], dtype=np.float32))
    bk = np.asarray(inputs["bk"], dtype=np.float32).reshape(1, C8)
    wv = np.ascontiguousarray(np.asarray(inputs["wv"], dtype=np.float32))
    bv = np.asarray(inputs["bv"], dtype=np.float32).reshape(1, C)
    gamma = np.asarray(inputs["gamma"], dtype=np.float32).reshape(1, 1)
    in_maps = []
    for b in range(B):
        in_maps.append(
            {
                "x": np.ascontiguousarray(x[b].reshape(N, C)),
                "wq": wq, "bq": bq, "wk": wk, "bk": bk,
                "wv": wv, "bv": bv, "gamma": gamma,
            }
        )
    return in_maps


def run(inputs, **kwargs):
    nc = _get_program()
    res = run_bass_kernel_spmd(
        nc, make_in_maps(inputs), core_ids=list(range(B)), **kwargs
    )
    y = np.stack([res.results[b]["y"] for b in range(B)], axis=0)
    return y.reshape(B, H, W, C).astype(np.float32), res


def kernel(**inputs) -> np.ndarray:
    y, _ = run(inputs)
    return y


# revision 7
# speedup vs baseline: 1.0796x; 1.0247x over previous
"""Trainium2 Bass kernel for per-sample conv self-attention.

Reference computation (per batch sample b, N = H*W = 4096, C = 64, C8 = 8):
    q = x @ wq + bq            [N, 8]
    k = x @ wk + bk            [N, 8]
    v = x @ wv + bv            [N, 64]
    attn = softmax(q @ k^T)    [N, N]   (softmax over keys, no scaling)
    out  = attn @ v * gamma + x

Sharding: data-parallel over batch — 8 samples onto 8 NeuronCores, one
sample per core.  Inside a core the attention matrix is processed
flash-style (never materialized in HBM):

  * x^T (with an appended ones-row for the biases) is built via PE
    transposes; q^T/k^T/v projections are small matmuls.
  * S^T blocks [m=128, n=512] are computed with K=8 matmuls packed 3-up
    into the 128x128 PE array via row groups (partition offsets 0/32/64),
    landing in PSUM banks.
  * exp() runs on ScalarE directly PSUM->SBUF (bf16 out) with a large
    free-dim (1536) to amortize per-instruction overhead.  No row-max
    subtraction is needed: |S| <= ~30 so exp stays well inside fp32/bf16
    range, matching the reference softmax up to rounding.
  * out^T [65, n] accumulates in PSUM as v'.T @ E^T where v' has a ones
    column appended - row 64 of out^T is then the softmax denominator.
  * Finale per n-block: cast to bf16, PE-transpose back to [n, 65],
    then VectorE computes y = out * (gamma/denom) + x and DMAs out.
"""

from contextlib import ExitStack

import numpy as np

import concourse.bass as bass
import concourse.mybir as mybir
import concourse.tile as tile
from concourse import bacc
from concourse.bass_utils import run_bass_kernel_spmd
from concourse.masks import make_identity

F32 = mybir.dt.float32
BF16 = mybir.dt.bfloat16
AF = mybir.ActivationFunctionType
ALU = mybir.AluOpType

B, H, W, C = 8, 64, 64, 64
N = H * W          # 4096 pixels (queries == keys)
C8 = C // 8        # 8  qk head dim
NB = 8             # n blocks
NBLK = N // NB     # 512 queries per block
MCH = N // 128     # 32 m-chunks of 128 keys
# m-chunks per group: 3 fit the 3 usable PE row-group replicas and give
# exp() free-dim 1536 (3 PSUM banks), double buffered: 6 banks + 1 out + 1 tp
GROUP_SIZES = [3] * 10 + [2]   # sums to 32


def _body(nc, tc, io):
    x_d, wq_d, bq_d, wk_d, bk_d, wv_d, bv_d, gamma_d, y_d = io


    # ---------------- persistent SBUF tensors ----------------
    ctx = ExitStack()
    singles = ctx.enter_context(tc.tile_pool(name="singles", bufs=1))
    ident = singles.tile([128, 128], BF16)       # identity for PE transposes
    x_sb = singles.tile([128, MCH * C], F32)     # resident x, chunk j at cols 64j
    xb = singles.tile([128, MCH * C], BF16)      # bf16 copy of x
    xT = singles.tile([C + 1, N], BF16)          # x^T with ones row 64
    qT_rep = singles.tile([128, N], BF16)        # q^T replicated at parts 0/32/64
    kT_rep = singles.tile([128, N], BF16)        # k^T replicated at parts 0/32/64
    v_all = singles.tile([128, MCH * (C + 1)], BF16)  # v'_j at cols 65j, ones col 64
    gamma_sb = singles.tile([128, 1], F32)
    wq_st = singles.tile([C + 1, C8], F32)
    wk_st = singles.tile([C + 1, C8], F32)
    wv_st = singles.tile([C + 1, C], F32)
    wqp = singles.tile([C + 1, 128], BF16)       # wq' replicated into cols 0/32/64
    wkp = singles.tile([C + 1, 128], BF16)
    wvp = singles.tile([C + 1, C], BF16)

    make_identity(nc, ident)

    # ---------------- input DMAs ----------------
    nc.sync.dma_start(out=wq_st[0:C, :], in_=wq_d)
    nc.sync.dma_start(out=wq_st[C : C + 1, :], in_=bq_d)
    nc.sync.dma_start(out=wk_st[0:C, :], in_=wk_d)
    nc.sync.dma_start(out=wk_st[C : C + 1, :], in_=bk_d)
    nc.sync.dma_start(out=wv_st[0:C, :], in_=wv_d)
    nc.sync.dma_start(out=wv_st[C : C + 1, :], in_=bv_d)
    nc.gpsimd.dma_start(out=gamma_sb[:], in_=gamma_d.to_broadcast((128, 1)))

    # weight staging -> bf16, replicated into PE column groups (zero padded)
    nc.gpsimd.memset(wqp[:], 0.0)
    nc.gpsimd.memset(wkp[:], 0.0)
    for i in range(3):
        nc.vector.tensor_copy(out=wqp[:, 32 * i : 32 * i + C8], in_=wq_st[:])
        nc.vector.tensor_copy(out=wkp[:, 32 * i : 32 * i + C8], in_=wk_st[:])
    nc.vector.tensor_copy(out=wvp[:], in_=wv_st[:])

    # ones row of x^T and ones columns of v'
    nc.gpsimd.memset(xT[C : C + 1, :], 1.0)
    nc.gpsimd.memset(
        v_all[:].rearrange("p (c f) -> p c f", f=C + 1)[:, :, C : C + 1], 1.0
    )

    # ---------------- preamble: transposes + projections ----------------
    # Pipelined in two n-halves so the main loop can start after half 0:
    # x DMA half -> bf16 cast -> PE transposes -> q/k projections (1024-wide
    # rounds, double buffered) -> v projections.  q casts go on ScalarE while
    # k casts go on VectorE so the two chains run concurrently.
    with ExitStack() as pre:
        pre_tp = pre.enter_context(tc.tile_pool(name="pre_tp", bufs=2, space="PSUM"))
        pre_v = pre.enter_context(tc.tile_pool(name="pre_v", bufs=2, space="PSUM"))
        pre_pj = pre.enter_context(tc.tile_pool(name="pre_pj", bufs=2, space="PSUM"))

        # quarter-granular pipeline: DMA -> cast -> 8 transposes -> q/k
        # projection round -> 8 v projections.  Quarter r feeds projection
        # round r exactly (cols 1024r..1024r+1023 = chunks 8r..8r+7), so the
        # main loop can start as soon as quarter 0 clears while quarters 1-3
        # stream behind it.
        for r in range(4):
            nc.sync.dma_start(
                out=x_sb[:, 512 * r : 512 * (r + 1)].rearrange(
                    "p (c f) -> p c f", f=C
                ),
                in_=x_d[1024 * r : 1024 * (r + 1), :].rearrange(
                    "(c p) f -> p c f", p=128
                ),
            )
            nc.vector.tensor_copy(
                out=xb[:, 512 * r : 512 * (r + 1)],
                in_=x_sb[:, 512 * r : 512 * (r + 1)],
            )
            for j in range(8 * r, 8 * r + 8):
                pt = pre_tp.tile([C, 128], BF16, tag="pt")
                nc.tensor.transpose(pt[:], xb[:, C * j : C * (j + 1)], ident[:])
                nc.vector.tensor_copy(
                    out=xT[0:C, 128 * j : 128 * (j + 1)], in_=pt[:]
                )
            csl = slice(1024 * r, 1024 * (r + 1))
            pq = pre_pj.tile([128, 1024], F32, tag="pj")
            for b2 in range(2):
                bsl = slice(1024 * r + 512 * b2, 1024 * r + 512 * (b2 + 1))
                nc.tensor.matmul(
                    pq[:, 512 * b2 : 512 * (b2 + 1)], wqp[:], xT[:, bsl],
                    start=True, stop=True,
                )
            nc.scalar.copy(out=qT_rep[:, csl], in_=pq[:])
            pk = pre_pj.tile([128, 1024], F32, tag="pj")
            for b2 in range(2):
                bsl = slice(1024 * r + 512 * b2, 1024 * r + 512 * (b2 + 1))
                nc.tensor.matmul(
                    pk[:, 512 * b2 : 512 * (b2 + 1)], wkp[:], xT[:, bsl],
                    start=True, stop=True,
                )
            nc.vector.tensor_copy(out=kT_rep[:, csl], in_=pk[:])
            for j in range(8 * r, 8 * r + 8):
                pv = pre_v.tile([128, C], F32, tag="pv")
                nc.tensor.matmul(
                    pv[:], xT[:, 128 * j : 128 * (j + 1)], wvp[:],
                    start=True, stop=True,
                )
                nc.vector.tensor_copy(
                    out=v_all[:, (C + 1) * j : (C + 1) * j + C], in_=pv[:]
                )

    # ---------------- main loop ----------------
    st_pool = ctx.enter_context(tc.tile_pool(name="st", bufs=2, space="PSUM"))
    out_pool = ctx.enter_context(tc.tile_pool(name="oacc", bufs=1, space="PSUM"))
    ytp_pool = ctx.enter_context(tc.tile_pool(name="ytp", bufs=1, space="PSUM"))
    et_pool = ctx.enter_context(tc.tile_pool(name="et", bufs=3))
    ob_pool = ctx.enter_context(tc.tile_pool(name="ob", bufs=2))
    fin_pool = ctx.enter_context(tc.tile_pool(name="fin", bufs=2))

    iters = []
    for nb in range(NB):
        j0 = 0
        for gi, gs in enumerate(GROUP_SIZES):
            iters.append((nb, gi, list(range(j0, j0 + gs))))
            j0 += gs

    oaccs = {}
    pending_finales = []  # (due_idx, nb, ob_tile)

    def emit_s_exp(nb, chunks):
        gw = NBLK * len(chunks)
        st = st_pool.tile([128, gw], F32, tag="st")
        nsl = slice(nb * NBLK, (nb + 1) * NBLK)
        for i, j in enumerate(chunks):
            nc.tensor.matmul(
                st[:, i * NBLK : (i + 1) * NBLK],
                kT_rep[32 * i : 32 * i + C8, 128 * j : 128 * (j + 1)],
                qT_rep[32 * i : 32 * i + C8, nsl],
                start=True, stop=True,
            )
        et = et_pool.tile([128, gw], BF16, tag="et")
        nc.scalar.activation(out=et[:], in_=st[:], func=AF.Exp)
        return et

    def emit_pv(nb, gi, chunks, et):
        if gi == 0:
            oaccs[nb] = out_pool.tile([128, NBLK], F32, tag="oacc", name=f"oacc{nb}")
        oacc = oaccs[nb]
        for i, j in enumerate(chunks):
            nc.tensor.matmul(
                oacc[0 : C + 1, :],
                v_all[:, (C + 1) * j : (C + 1) * (j + 1)],
                et[:, i * NBLK : (i + 1) * NBLK],
                start=(j == 0), stop=(j == MCH - 1),
                skip_group_check=True,
            )

    def emit_finale(nb, ob, k4):
        if True:
            yt = ytp_pool.tile([128, C + 1], BF16, tag="yt")
            nc.tensor.transpose(
                yt[:], ob[:, 128 * k4 : 128 * (k4 + 1)], ident[0 : C + 1, 0 : C + 1]
            )
            rc = fin_pool.tile([128, 1], F32, tag="rc")
            nc.vector.reciprocal(rc[:], yt[:, C : C + 1])
            yo = fin_pool.tile([128, C], F32, tag="yo")
            nc.vector.tensor_scalar(
                out=yo[:], in0=yt[:, 0:C], scalar1=rc[:], scalar2=gamma_sb[:],
                op0=ALU.mult, op1=ALU.mult,
            )
            ck = nb * 4 + k4
            nc.vector.tensor_add(yo[:], yo[:], x_sb[:, C * ck : C * (ck + 1)])
            nc.sync.dma_start(out=y_d[128 * ck : 128 * (ck + 1), :], in_=yo[:])

    prevs = []
    for idx, (nb, gi, chunks) in enumerate(iters):
        while pending_finales and pending_finales[0][0] <= idx:
            _, fnb, fob, fk4 = pending_finales.pop(0)
            emit_finale(fnb, fob, fk4)
        et = emit_s_exp(nb, chunks)
        prevs.append((nb, gi, chunks, et))
        if len(prevs) > 2:
            pnb, pgi, pchunks, pet = prevs.pop(0)
            emit_pv(pnb, pgi, pchunks, pet)
            if pgi == len(GROUP_SIZES) - 1:
                ob = ob_pool.tile([C + 1, NBLK], BF16, tag="ob")
                nc.vector.tensor_copy(out=ob[:], in_=oaccs[pnb][0 : C + 1, :])
                for t in range(4):
                    pending_finales.append((idx + 1 + t, pnb, ob, t))

    for pnb, pgi, pchunks, pet in prevs:
        emit_pv(pnb, pgi, pchunks, pet)
        if pgi == len(GROUP_SIZES) - 1:
            ob = ob_pool.tile([C + 1, NBLK], BF16, tag="ob")
            nc.vector.tensor_copy(out=ob[:], in_=oaccs[pnb][0 : C + 1, :])
            for t in range(4):
                pending_finales.append((0, pnb, ob, t))
    for _, fnb, fob, fk4 in pending_finales:
        emit_finale(fnb, fob, fk4)

    ctx.close()


def build_program():
    nc = bacc.Bacc("TRN2", target_bir_lowering=False, debug=False, num_devices=8)
    x_d = nc.dram_tensor("x", [N, C], F32, kind="ExternalInput").ap()
    wq_d = nc.dram_tensor("wq", [C, C8], F32, kind="ExternalInput").ap()
    bq_d = nc.dram_tensor("bq", [1, C8], F32, kind="ExternalInput").ap()
    wk_d = nc.dram_tensor("wk", [C, C8], F32, kind="ExternalInput").ap()
    bk_d = nc.dram_tensor("bk", [1, C8], F32, kind="ExternalInput").ap()
    wv_d = nc.dram_tensor("wv", [C, C], F32, kind="ExternalInput").ap()
    bv_d = nc.dram_tensor("bv", [1, C], F32, kind="ExternalInput").ap()
    gamma_d = nc.dram_tensor("gamma", [1, 1], F32, kind="ExternalInput").ap()
    y_d = nc.dram_tensor("y", [N, C], F32, kind="ExternalOutput").ap()

    io = (x_d, wq_d, bq_d, wk_d, bk_d, wv_d, bv_d, gamma_d, y_d)
    with tile.TileContext(nc) as tc:
        _body(nc, tc, io)
    nc.compile()
    return nc


_CACHE = {}


def _get_program():
    if "nc" not in _CACHE:
        _CACHE["nc"] = build_program()
    return _CACHE["nc"]


def make_in_maps(inputs):
    x = np.ascontiguousarray(np.asarray(inputs["x"], dtype=np.float32))
    wq = np.ascontiguousarray(np.asarray(inputs["wq"], dtype=np.float32))
    bq = np.asarray(inputs["bq"], dtype=np.float32).reshape(1, C8)
    wk = np.ascontiguousarray(np.asarray(inputs["wk"], dtype=np.float32))
    bk = np.asarray(inputs["bk"], dtype=np.float32).reshape(1, C8)
    wv = np.ascontiguousarray(np.asarray(inputs["wv"], dtype=np.float32))
    bv = np.asarray(inputs["bv"], dtype=np.float32).reshape(1, C)
    gamma = np.asarray(inputs["gamma"], dtype=np.float32).reshape(1, 1)
    in_maps = []
    for b in range(B):
        in_maps.append(
            {
                "x": np.ascontiguousarray(x[b].reshape(N, C)),
                "wq": wq, "bq": bq, "wk": wk, "bk": bk,
                "wv": wv, "bv": bv, "gamma": gamma,
            }
        )
    return in_maps


def run(inputs, **kwargs):
    nc = _get_program()
    res = run_bass_kernel_spmd(
        nc, make_in_maps(inputs), core_ids=list(range(B)), **kwargs
    )
    y = np.stack([res.results[b]["y"] for b in range(B)], axis=0)
    return y.reshape(B, H, W, C).astype(np.float32), res


def kernel(**inputs) -> np.ndarray:
    y, _ = run(inputs)
    return y
